# revision 3
# baseline (speedup 1.0000x reference)
"""Trainium2 Bass kernel for fake-quant (W8A8) linear: y = fq_tok(x) @ fq_ch(w).T + b.

Full shapes: x [4, 2048, 4096] f32, w [4096, 4096] f32, b [4096] f32.
Sharding over 8 cores: 2 token groups x 4 out-channel groups.
Per core: x_sh [4096, 4096], w_sh [1024, 4096], b_sh [1024] -> y_sh [4096, 1024].

Quantized values are integers in [-127, 127], exactly representable in bf16,
so the matmul runs on the PE array in bf16 (full rate) with fp32 PSUM
accumulation - numerically equivalent to the fp32 reference einsum on the
dequantized values.  Scales are applied in the fp32 epilogue.

v2 over the 642us baseline: all 128x128 transposes move off the PE onto the
DMA xbar (`dma_start(transpose=True)`, one 1MB transpose per token tile;
out[p, kb, t] = in[t, kb*128+p] matches the 3D qxT layout directly), so the
PE does nothing but the 2048 N=512 bf16 matmuls, which stream at the 216 ns
roofline.  Startup is restructured: qwT is split into two 512-channel halves
so tile 0's cb0 matmuls only wait on w blocks 0-3, the first three x tiles
are quantized while w blocks 4-7 are still in flight, and the PE queue is
emitted cb0-first across tiles 0-2 (FIFO order would otherwise block tile 1
behind tile 0's cb1 wait on qwT half 1).

Rounding: round-half-to-even via the fp32 magic-constant trick
(v + 1.5*2^23 rounds mantissa to integer; subtract again afterwards),
matching jnp.round.  Clipping to [-128, 127] is a no-op by construction
(|x|/s <= 127 when s = amax/127) so it is skipped.

Engine split: DVE does amax + scale/reciprocal + the fp32 epilogue
(psum*sx*sw, +bias); ACT does the two rounding passes (f32 magic-add, then
magic-subtract to bf16); Sync issues all DMAs including the transposes;
PE does only matmuls.
"""

from contextlib import ExitStack

import numpy as np

import concourse.bass as bass
import concourse.mybir as mybir
import concourse.tile as tile
from concourse import bacc

P = 128
MAGIC = 12582912.0  # 1.5 * 2**23
QMAX = 127.0
EPS = 1e-8

# full problem shapes (hardcoded per harness contract)
B, S, D_IN, D_OUT = 4, 2048, 4096, 4096
TOK = B * S  # 8192
TOK_GROUPS = 2
CH_GROUPS = 4
T_SH = TOK // TOK_GROUPS  # 4096 tokens per core
O_SH = D_OUT // CH_GROUPS  # 1024 channels per core


def build_nc(T, K, O, nch=512):
    """Build the per-core Bass program: x[T,K], w[O,K], b[O] -> y[T,O]."""
    f32 = mybir.dt.float32
    bf16 = mybir.dt.bfloat16
    Copy = mybir.ActivationFunctionType.Copy
    Alu = mybir.AluOpType
    AxX = mybir.AxisListType.X

    assert T % P == 0 and K % P == 0 and O % P == 0
    TT, KB, WT = T // P, K // P, O // P
    NCH = min(nch, O)
    CB = O // NCH
    assert CB == 2 and WT == 8, "startup interleave assumes 2 halves x 4 blocks"
    WPH = WT // CB  # w blocks per qwT half

    nc = bacc.Bacc("TRN2", target_bir_lowering=False, debug=False)
    x_ap = nc.dram_tensor("x", [T, K], f32, kind="ExternalInput").ap()
    w_ap = nc.dram_tensor("w", [O, K], f32, kind="ExternalInput").ap()
    b_ap = nc.dram_tensor("b", [O], f32, kind="ExternalInput").ap()
    y_ap = nc.dram_tensor("y", [T, O], f32, kind="ExternalOutput").ap()

    with tile.TileContext(nc) as tc, ExitStack() as ctx:
        singles = ctx.enter_context(tc.tile_pool(name="singles", bufs=1))
        bigf32 = ctx.enter_context(tc.tile_pool(name="bigf32", bufs=3))
        rnd = ctx.enter_context(tc.tile_pool(name="rnd", bufs=1))
        qpool = ctx.enter_context(tc.tile_pool(name="qpool", bufs=3))
        qtpool = ctx.enter_context(tc.tile_pool(name="qtpool", bufs=3))
        stats = ctx.enter_context(tc.tile_pool(name="stats", bufs=8))
        sxpool = ctx.enter_context(tc.tile_pool(name="sxpool", bufs=5))
        opool = ctx.enter_context(tc.tile_pool(name="opool", bufs=4))
        psum_pool = ctx.enter_context(tc.tile_pool(name="psum", bufs=6, space="PSUM"))
        dram = ctx.enter_context(tc.tile_pool(name="dram", bufs=1, space="DRAM"))

        # resident: transposed quantized weights (two 512-ch halves so cb0
        # matmuls only depend on w blocks 0-3) + broadcast scale/bias rows
        qwT = [singles.tile([P, KB, NCH], bf16, name=f"qwT{h}") for h in range(CB)]
        sw_b = singles.tile([P, O], f32)
        bb_b = singles.tile([P, O], f32)
        sw_dram = dram.tile([O, 1], f32)

        def quantize(src_t, q_t, s_t):
            # per-row amax -> scale (s_t), then round src*(1/s) to q_t (bf16)
            amax = stats.tile([P, 1], f32, tag="st", name="amax")
            nc.vector.reduce_max(
                out=amax, in_=src_t, axis=AxX, apply_absolute_value=True
            )
            nc.vector.tensor_scalar(
                out=s_t, in0=amax, scalar1=1.0 / QMAX, scalar2=EPS,
                op0=Alu.mult, op1=Alu.max,
            )
            r_t = stats.tile([P, 1], f32, tag="st", name="recip")
            nc.vector.reciprocal(out=r_t, in_=s_t)
            t_t = rnd.tile([P, K], f32, tag="rnd", name="t_round")
            # round on ACT (scale is a per-partition pointer operand; the
            # Bacc event-semaphore pass legalizes its single-wait limit)
            nc.scalar.activation(
                out=t_t, in_=src_t, func=Copy, bias=MAGIC, scale=r_t[:, 0:1]
            )
            nc.scalar.activation(out=q_t, in_=t_t, func=Copy, bias=-MAGIC, scale=1.0)

        # ---- per-block / per-tile stages ----
        def w_block(wt):
            w_t = bigf32.tile([P, K], f32, tag="big", name=f"w_{wt}")
            nc.sync.dma_start(out=w_t, in_=w_ap[wt * P : (wt + 1) * P, :])
            sw = stats.tile([P, 1], f32, tag="st", name=f"sw_{wt}")
            qw = qpool.tile([P, K], bf16, tag="q", name=f"qw_{wt}")
            quantize(w_t, qw, sw)
            h, c = divmod(wt, WPH)
            nc.sync.dma_start(
                out=qwT[h][:, :, c * P : (c + 1) * P], in_=qw, transpose=True
            )
            nc.sync.dma_start(out=sw_dram[wt * P : (wt + 1) * P, :], in_=sw)

        def load_x(tt):
            x_t = bigf32.tile([P, K], f32, tag="big", name=f"x_{tt}")
            nc.sync.dma_start(out=x_t, in_=x_ap[tt * P : (tt + 1) * P, :])
            return x_t

        def quant_x(tt, x_t):
            sx = sxpool.tile([P, 1], f32, tag="sx", name=f"sx_{tt}")
            qx = qpool.tile([P, K], bf16, tag="q", name=f"qx_{tt}")
            quantize(x_t, qx, sx)
            qxT = qtpool.tile([P, KB, P], bf16)  # qxT[f, k, t] = qx[t, k*128+f]
            nc.sync.dma_start(out=qxT, in_=qx, transpose=True)
            return sx, qxT

        def mm_group(tt, cb, sx, qxT):
            ps = psum_pool.tile([P, NCH], f32, tag="psum", name=f"ps_{tt}_{cb}")
            for k in range(KB):
                nc.tensor.matmul(
                    ps,
                    qxT[:, k, :],
                    qwT[cb][:, k, :],
                    start=(k == 0),
                    stop=(k == KB - 1),
                )
            return ps

        def epilogue(tt, cb, sx, ps):
            o1 = opool.tile([P, NCH], f32, tag="o", name=f"o1_{tt}_{cb}")
            nc.vector.scalar_tensor_tensor(
                out=o1, in0=ps, scalar=sx[:, 0:1],
                in1=sw_b[:, cb * NCH : (cb + 1) * NCH],
                op0=Alu.mult, op1=Alu.mult,
            )
            o2 = opool.tile([P, NCH], f32, tag="o", name=f"o2_{tt}_{cb}")
            nc.vector.tensor_add(
                out=o2, in0=o1, in1=bb_b[:, cb * NCH : (cb + 1) * NCH]
            )
            nc.sync.dma_start(
                out=y_ap[tt * P : (tt + 1) * P, cb * NCH : (cb + 1) * NCH], in_=o2
            )

        # ---- startup: interleave w blocks with the first x tiles ----
        NPRE = 3  # x tiles quantized during the w phase
        x_tiles = {}
        x_tiles[0] = load_x(0)
        for wt in range(2):
            w_block(wt)
        xq = {}
        xq[0] = quant_x(0, x_tiles[0])
        for wt in range(2, WPH):
            w_block(wt)
        x_tiles[1] = load_x(1)
        w_block(WPH)
        xq[1] = quant_x(1, x_tiles[1])
        w_block(WPH + 1)
        x_tiles[2] = load_x(2)
        xq[2] = quant_x(2, x_tiles[2])
        for wt in range(WPH + 2, WT):
            w_block(wt)

        # broadcast per-channel scale & bias across partitions
        nc.sync.dma_start(
            out=sw_b,
            in_=bass.AP(tensor=sw_dram.tensor, offset=sw_dram.offset, ap=[[0, P], [1, O]]),
        )
        nc.sync.dma_start(
            out=bb_b,
            in_=bass.AP(tensor=b_ap.tensor, offset=b_ap.offset, ap=[[0, P], [1, O]]),
        )

        # ---- PE ramp: cb0 groups of tiles 0..2 first (they only need qwT
        # half 0), then their cb1 groups, then steady state ----
        pend = {}
        for tt in range(NPRE):
            pend[(tt, 0)] = mm_group(tt, 0, *xq[tt])
        for tt in range(NPRE):
            epilogue(tt, 0, xq[tt][0], pend.pop((tt, 0)))
            pend[(tt, 1)] = mm_group(tt, 1, *xq[tt])
        for tt in range(NPRE):
            epilogue(tt, 1, xq[tt][0], pend.pop((tt, 1)))

        # ---- steady state ----
        for tt in range(NPRE, TT):
            x_t = load_x(tt)
            sx, qxT = quant_x(tt, x_t)
            for cb in range(CB):
                ps = mm_group(tt, cb, sx, qxT)
                epilogue(tt, cb, sx, ps)
    nc.compile()
    return nc


_cached_nc = None


def _get_nc():
    global _cached_nc
    if _cached_nc is None:
        _cached_nc = build_nc(T_SH, D_IN, O_SH)
    return _cached_nc


def kernel(x: np.ndarray, w: np.ndarray, b: np.ndarray, _trace=False):
    from concourse.bass_utils import run_bass_kernel_spmd

    assert x.shape == (B, S, D_IN) and w.shape == (D_OUT, D_IN) and b.shape == (D_OUT,)
    x2 = np.ascontiguousarray(x.reshape(TOK, D_IN), dtype=np.float32)
    w2 = np.ascontiguousarray(w, dtype=np.float32)
    b2 = np.ascontiguousarray(b, dtype=np.float32)

    in_maps = []
    for core in range(8):
        tg, cg = divmod(core, CH_GROUPS)
        in_maps.append(
            {
                "x": np.ascontiguousarray(x2[tg * T_SH : (tg + 1) * T_SH]),
                "w": np.ascontiguousarray(w2[cg * O_SH : (cg + 1) * O_SH]),
                "b": np.ascontiguousarray(b2[cg * O_SH : (cg + 1) * O_SH]),
            }
        )

    nc = _get_nc()
    res = run_bass_kernel_spmd(nc, in_maps, core_ids=list(range(8)), trace=_trace)

    y = np.empty((TOK, D_OUT), dtype=np.float32)
    for core in range(8):
        tg, cg = divmod(core, CH_GROUPS)
        y[tg * T_SH : (tg + 1) * T_SH, cg * O_SH : (cg + 1) * O_SH] = res.results[
            core
        ]["y"]
    if _trace:
        kernel._last_results = res
    return y.reshape(B, S, D_OUT)
